# revision 28
# baseline (speedup 1.0000x reference)
"""AGNNConv message-passing kernel for 8 TRN2 NeuronCores.

Sharding: edges sorted by destination; core c owns dst nodes
[12500c, 12500(c+1)).  Within a core, edges are grouped by
(dst-block of 128 nodes, src-chunk of 25088 nodes) and padded to a
multiple of 128; group sizes are uniform across cores (max over cores)
so all 8 cores run one SPMD graph.

Device graph per core:
  Phase A: L2-normalize the full node table to bf16 (4 src-chunk tables
           + own-dst-range table).
  Phase B: per piece, dma_gather src rows (S) and dst rows (Y);
           cos = reduce(S*Y); w = exp(beta*cos); masked one-hot matmuls
           accumulate num^T[d,j] and s[j] per dst block in PSUM;
           finalize out[j,:] = num[j,:] / s[j].
"""

import sys

sys.path.insert(0, "/opt/trn_rl_repo")

import ml_dtypes
import numpy as np

import concourse.bass as bass
import concourse.mybir as mybir
import concourse.tile as tile
from concourse import bacc
from concourse.bass_utils import run_bass_kernel_spmd
from concourse.library_config import mlp
from concourse.masks import make_identity

N, E, D = 100000, 3200000, 128
P = 128
NCORES = 8
NPC = 12500          # real dst nodes per core
NB = 98              # dst blocks per core (98*128 = 12544)
NPC_PAD = NB * P
NCHUNK = 4
CHUNK = 25088        # src chunk rows (196*128, < 2**15)
EPS = 1e-12
BPP = 3              # dst blocks per gather piece
UMAX = 16            # units (128 edges) per vector op group
F32 = mybir.dt.float32
BF16 = mybir.dt.bfloat16
I16 = mybir.dt.int16


def _pack_idx(arr_i16):
    """dma_gather index layout: idx i -> [i % 16, i // 16], replicated to
    128 partitions (8 copies of the 16-partition pattern)."""
    n = arr_i16.shape[0]
    l = arr_i16.reshape(n // 16, 16).T  # [16, n/16]
    return np.tile(l, (8, 1))  # [128, n/16]


def _prep(feat, beta, src, dst):
    """Host-side shard/index prep. Returns (in_maps, static) where static
    holds the compile-time structure shared by all cores."""
    nrm = np.sqrt((feat.astype(np.float32) ** 2).sum(1))  # [N]
    nrm = np.maximum(nrm, EPS)

    order = np.argsort(dst, kind="stable")
    ss = src[order].astype(np.int64)
    dd = dst[order].astype(np.int64)

    core = dd // NPC
    within = dd % NPC
    block = within // P
    jloc = (within % P).astype(np.float32)
    chunk = ss // CHUNK
    slocal = (ss % CHUNK).astype(np.int16)
    ylocal = within.astype(np.int16)

    comp = ((core * NB + block) * NCHUNK + chunk).astype(np.int64)
    counts = np.bincount(comp, minlength=NCORES * NB * NCHUNK).reshape(
        NCORES, NB, NCHUNK
    )
    Tbk = (np.ceil(counts.max(axis=0) / P) * P).astype(np.int64)  # [NB, NCHUNK]
    STk = Tbk.sum(axis=0)  # [NCHUNK] per-chunk stream length
    ST = int(STk.sum())
    NU = ST // P

    # edge order inside each (core, block, chunk) group
    order2 = np.argsort(comp, kind="stable")
    ss2, jl2, ch2, sl2, yl2 = (
        ss[order2],
        jloc[order2],
        chunk[order2],
        slocal[order2],
        ylocal[order2],
    )
    nrm_src = nrm[ss2].astype(np.float32)
    # start offset of each (c,b,k) group in the sorted edge array
    gstart = np.zeros(NCORES * NB * NCHUNK + 1, dtype=np.int64)
    np.cumsum(counts.reshape(-1), out=gstart[1:])

    # stream offsets: chunk-major streams, blocks in order inside each
    off_k = np.zeros((NCHUNK, NB + 1), dtype=np.int64)
    for k in range(NCHUNK):
        np.cumsum(Tbk[:, k], out=off_k[k, 1:])
    chunk_base = np.zeros(NCHUNK + 1, dtype=np.int64)
    np.cumsum(STk, out=chunk_base[1:])

    in_maps = []
    for c in range(NCORES):
        sidx = np.zeros(ST, dtype=np.int16)
        yidx = np.zeros(ST, dtype=np.int16)
        dloc = np.full(ST, 999.0, dtype=np.float32)
        wnrm = np.zeros(ST, dtype=np.float32)
        for k in range(NCHUNK):
            for b in range(NB):
                g = (c * NB + b) * NCHUNK + k
                cnt = gstart[g + 1] - gstart[g]
                if cnt == 0:
                    continue
                o = chunk_base[k] + off_k[k, b]
                sl_ = slice(gstart[g], gstart[g + 1])
                sidx[o : o + cnt] = sl2[sl_]
                yidx[o : o + cnt] = yl2[sl_]
                dloc[o : o + cnt] = jl2[sl_]
                wnrm[o : o + cnt] = 1.0 / nrm_src[sl_]

        lo = c * NPC
        fown = np.zeros((NPC_PAD, D), dtype=np.float32)
        fown[:NPC] = feat[lo : lo + NPC]

        in_maps.append(
            {
                "feat": np.ascontiguousarray(feat.astype(np.float32)),
                "fown": fown,
                "sidx": np.ascontiguousarray(_pack_idx(sidx)),
                "yidx": np.ascontiguousarray(_pack_idx(yidx)),
                "dloc": np.ascontiguousarray(dloc.reshape(NU, P).T),
                "wnrm": np.ascontiguousarray(wnrm.reshape(NU, P).T),
                "iota": np.ascontiguousarray(
                    np.tile(np.arange(P, dtype=np.float32), (P, 1))
                ).astype(ml_dtypes.bfloat16),
                "betar": np.full((P, 1), float(beta[0]), dtype=np.float32),
            }
        )

    static = dict(Tbk=Tbk, off_k=off_k, chunk_base=chunk_base, ST=ST, NU=NU)
    return in_maps, static


def _build(static):
    import os

    BISECT = os.environ.get("KBISECT", "")

    Tbk = static["Tbk"]
    off_k = static["off_k"]
    chunk_base = static["chunk_base"]
    ST = static["ST"]
    NU = static["NU"]

    nc = bacc.Bacc("TRN2")

    feat_ext = nc.declare_dram_parameter("feat", [N, D], F32, isOutput=False)
    fown_ext = nc.declare_dram_parameter("fown", [NPC_PAD, D], F32, isOutput=False)
    sidx_ext = nc.declare_dram_parameter("sidx", [P, ST // 16], I16, isOutput=False)
    yidx_ext = nc.declare_dram_parameter("yidx", [P, ST // 16], I16, isOutput=False)
    dloc_ext = nc.declare_dram_parameter("dloc", [P, NU], F32, isOutput=False)
    wnrm_ext = nc.declare_dram_parameter("wnrm", [P, NU], F32, isOutput=False)
    iota_ext = nc.declare_dram_parameter("iota", [P, P], BF16, isOutput=False)
    betar_ext = nc.declare_dram_parameter("betar", [P, 1], F32, isOutput=False)
    out_ext = nc.declare_dram_parameter("out", [NPC_PAD, D], F32, isOutput=True)

    htabs = [
        nc.dram_tensor(f"htab{k}", [CHUNK, D], BF16) for k in range(NCHUNK)
    ]
    hown = nc.dram_tensor("hown", [NPC_PAD, D], BF16)

    # piece structure over blocks
    pieces = [(b0, min(b0 + BPP, NB)) for b0 in range(0, NB, BPP)]
    LPK = np.zeros((len(pieces), NCHUNK), dtype=np.int64)
    for pi, (b0, b1) in enumerate(pieces):
        for k in range(NCHUNK):
            LPK[pi, k] = Tbk[b0:b1, k].sum()
    LMAX = int(LPK.max())

    with tile.TileContext(nc) as tc:
        with (
            tc.tile_pool(name="consts", bufs=1) as cpool,
            tc.tile_pool(name="pa", bufs=2) as papool,
            tc.tile_pool(name="pan", bufs=2) as pnpool,
            tc.tile_pool(name="gat", bufs=8) as gpool,
            tc.tile_pool(name="idx", bufs=8) as ipool,
            tc.tile_pool(name="vec", bufs=3) as vpool,
            tc.tile_pool(name="mw", bufs=4) as mpool,
            tc.tile_pool(name="fin", bufs=2) as fpool,
            tc.tile_pool(name="ps", bufs=2, space="PSUM") as pspool,
            tc.tile_pool(name="ps2", bufs=2, space="PSUM") as ps2pool,
        ):
            nc.gpsimd.load_library(mlp)

            iota_t = cpool.tile([P, P], BF16)
            nc.sync.dma_start(out=iota_t[:], in_=iota_ext[:])
            betar_t = cpool.tile([P, 1], F32)
            nc.sync.dma_start(out=betar_t[:], in_=betar_ext[:])
            ones_t = cpool.tile([P, 1], BF16)
            nc.vector.memset(ones_t[:], 1.0)
            ident_t = cpool.tile([P, P], F32)
            make_identity(nc, ident_t[:])

            # ---------------- Phase A: build bf16 tables ----------------
            # Rows are mapped partition-contiguously: partition p holds rows
            # [a + p*gn, a + (p+1)*gn) — 1 contiguous descriptor/partition.
            GA = 16  # row-tiles (x128 rows) per batched DMA

            def cast_range(src_ap, dst_ap, nrows):
                """plain f32 -> bf16 cast of nrows rows (raw src table)."""
                done = 0
                while done < nrows:
                    gn = min(GA, (nrows - done) // P)
                    if gn >= 1:
                        rows = gn * P
                        ft = papool.tile([P, GA, P], F32)
                        nc.sync.dma_start(
                            out=ft[:, :gn, :],
                            in_=src_ap[done : done + rows, :].rearrange(
                                "(p q) d -> p q d", p=P
                            ),
                        )
                        ht = papool.tile([P, GA, P], BF16)
                        nc.vector.tensor_copy(out=ht[:, :gn, :], in_=ft[:, :gn, :])
                        nc.scalar.dma_start(
                            out=dst_ap[done : done + rows, :].rearrange(
                                "(p q) d -> p q d", p=P
                            ),
                            in_=ht[:, :gn, :],
                        )
                        done += rows
                    else:
                        rem = nrows - done
                        ft = papool.tile([P, P], F32)
                        nc.sync.dma_start(
                            out=ft[:rem, :], in_=src_ap[done:nrows, :]
                        )
                        ht = papool.tile([P, P], BF16)
                        nc.vector.tensor_copy(out=ht[:rem, :], in_=ft[:rem, :])
                        nc.scalar.dma_start(
                            out=dst_ap[done:nrows, :], in_=ht[:rem, :]
                        )
                        done = nrows

            def conv_range(src_ap, dst_ap, nrows):
                """L2-normalize nrows rows (nrows % 128 == 0) to bf16."""
                done = 0
                while done < nrows:
                    gn = min(GA, (nrows - done) // P)
                    rows = gn * P
                    ft = papool.tile([P, GA, P], F32)
                    nc.sync.dma_start(
                        out=ft[:, :gn, :],
                        in_=src_ap[done : done + rows, :].rearrange(
                            "(p q) d -> p q d", p=P
                        ),
                    )
                    sqt = papool.tile([P, GA, P], F32)
                    nc.vector.tensor_tensor(
                        out=sqt[:, :gn, :],
                        in0=ft[:, :gn, :],
                        in1=ft[:, :gn, :],
                        op=mybir.AluOpType.mult,
                    )
                    sqcols = pnpool.tile([P, GA], F32)
                    nc.vector.tensor_reduce(
                        out=sqcols[:, :gn],
                        in_=sqt[:, :gn, :],
                        axis=mybir.AxisListType.X,
                        op=mybir.AluOpType.add,
                    )
                    nrmc = pnpool.tile([P, GA], F32)
                    nc.scalar.sqrt(out=nrmc[:, :gn], in_=sqcols[:, :gn])
                    invc = pnpool.tile([P, GA], F32)
                    nc.vector.tensor_scalar_max(
                        out=nrmc[:, :gn], in0=nrmc[:, :gn], scalar1=EPS
                    )
                    nc.vector.reciprocal(out=invc[:, :gn], in_=nrmc[:, :gn])
                    ht = papool.tile([P, GA, P], BF16)
                    for gi in range(gn):
                        nc.vector.tensor_scalar(
                            out=ht[:, gi, :],
                            in0=ft[:, gi, :],
                            scalar1=invc[:, gi : gi + 1],
                            scalar2=None,
                            op0=mybir.AluOpType.mult,
                        )
                    nc.scalar.dma_start(
                        out=dst_ap[done : done + rows, :].rearrange(
                            "(p q) d -> p q d", p=P
                        ),
                        in_=ht[:, :gn, :],
                    )
                    done += rows

            if BISECT != "z":
                for k in range(NCHUNK):
                    lo = k * CHUNK
                    nrows = min(CHUNK, N - lo)
                    cast_range(feat_ext[lo : lo + nrows, :], htabs[k][:], nrows)
                conv_range(fown_ext[:], hown[:], NPC_PAD)

            # ---------------- Phase B: edges ----------------
            # (Phase A -> B ordering is enforced by shadow-memory DRAM deps)
            if BISECT:
                zt = cpool.tile([P, P], F32)
                nc.vector.memset(zt[:], 0.0)
                for b in range(NB):
                    nc.sync.dma_start(
                        out=out_ext[b * P : (b + 1) * P, :], in_=zt[:]
                    )
            piece_list = [] if BISECT in ("a", "z") else pieces
            for pi, (b0, b1) in enumerate(piece_list):
                Sts, Yts, dlts, wnts = [], [], [], []
                ubase_piece = []
                for k in range(NCHUNK):
                    ln = int(LPK[pi, k])
                    if ln == 0:
                        Sts.append(None)
                        Yts.append(None)
                        dlts.append(None)
                        wnts.append(None)
                        ubase_piece.append(0)
                        continue
                    lu = ln // P
                    c0 = int((chunk_base[k] + off_k[k, b0]) // 16)
                    u0 = int((chunk_base[k] + off_k[k, b0]) // P)
                    ubase_piece.append(u0)
                    si = ipool.tile([P, LMAX // 16], I16)
                    nc.sync.dma_start(
                        out=si[:, : ln // 16], in_=sidx_ext[:, c0 : c0 + ln // 16]
                    )
                    yi = ipool.tile([P, LMAX // 16], I16)
                    nc.sync.dma_start(
                        out=yi[:, : ln // 16], in_=yidx_ext[:, c0 : c0 + ln // 16]
                    )
                    dl = ipool.tile([P, LMAX // P], F32)
                    nc.sync.dma_start(out=dl[:, :lu], in_=dloc_ext[:, u0 : u0 + lu])
                    wn = ipool.tile([P, LMAX // P], F32)
                    nc.sync.dma_start(out=wn[:, :lu], in_=wnrm_ext[:, u0 : u0 + lu])
                    St = gpool.tile([P, LMAX // P, P], BF16)
                    nc.gpsimd.dma_gather(
                        St[:, :lu, :],
                        htabs[k][:],
                        si[:, : ln // 16],
                        ln,
                        ln,
                        D,
                        single_packet=False,
                    )
                    Yt = gpool.tile([P, LMAX // P, P], BF16)
                    nc.gpsimd.dma_gather(
                        Yt[:, :lu, :],
                        hown[:],
                        yi[:, : ln // 16],
                        ln,
                        ln,
                        D,
                        single_packet=False,
                    )
                    Sts.append(St)
                    Yts.append(Yt)
                    dlts.append(dl)
                    wnts.append(wn)

                if BISECT == "g":
                    for k in range(NCHUNK):
                        if Sts[k] is not None:
                            nc.vector.tensor_copy(
                                out=Sts[k][:, 0, :], in_=Yts[k][:, 0, :]
                            )
                    continue
                for b in range(b0, b1):
                    numT = pspool.tile([P, P], F32)
                    scol = ps2pool.tile([P, 1], F32)
                    # count matmuls in this block for start/stop flags
                    nmm = sum(int(Tbk[b, k]) // P for k in range(NCHUNK))
                    mi = 0
                    for k in range(NCHUNK):
                        nub = int(Tbk[b, k]) // P
                        if nub == 0:
                            continue
                        St, Yt, dl, wn = Sts[k], Yts[k], dlts[k], wnts[k]
                        ub = int((chunk_base[k] + off_k[k, b]) // P) - ubase_piece[k]
                        for ug in range(0, nub, UMAX):
                            un = min(UMAX, nub - ug)
                            o = ub + ug
                            prod = vpool.tile([P, UMAX, P], BF16)
                            nc.vector.tensor_tensor(
                                out=prod[:, :un, :],
                                in0=St[:, o : o + un, :],
                                in1=Yt[:, o : o + un, :],
                                op=mybir.AluOpType.mult,
                            )
                            dot = vpool.tile([P, UMAX], F32)
                            nc.vector.tensor_reduce(
                                out=dot[:, :un],
                                in_=prod[:, :un, :],
                                axis=mybir.AxisListType.X,
                                op=mybir.AluOpType.add,
                            )
                            t2 = vpool.tile([P, UMAX], F32)
                            nc.vector.tensor_tensor(
                                out=t2[:, :un],
                                in0=dot[:, :un],
                                in1=wn[:, o : o + un],
                                op=mybir.AluOpType.mult,
                            )
                            wexp = vpool.tile([P, UMAX], F32)
                            nc.scalar.activation(
                                out=wexp[:, :un],
                                in_=t2[:, :un],
                                func=mybir.ActivationFunctionType.Exp,
                                scale=betar_t[:, 0:1],
                            )
                            for u in range(un):
                                mw = mpool.tile([P, P], BF16)
                                nc.vector.tensor_scalar(
                                    out=mw[:],
                                    in0=iota_t[:],
                                    scalar1=dl[:, o + u : o + u + 1],
                                    scalar2=wexp[:, u : u + 1],
                                    op0=mybir.AluOpType.is_equal,
                                    op1=mybir.AluOpType.mult,
                                )
                                nc.tensor.matmul(
                                    out=numT[:],
                                    lhsT=St[:, o + u, :],
                                    rhs=mw[:],
                                    start=(mi == 0),
                                    stop=(mi == nmm - 1),
                                    skip_group_check=True,
                                )
                                nc.tensor.matmul(
                                    out=scol[:],
                                    lhsT=mw[:],
                                    rhs=ones_t[:],
                                    start=(mi == 0),
                                    stop=(mi == nmm - 1),
                                    skip_group_check=True,
                                )
                                mi += 1
                    # finalize block b
                    numS = fpool.tile([P, P], F32)
                    nc.scalar.copy(out=numS[:], in_=numT[:])
                    outT = pspool.tile([P, P], F32)
                    nc.tensor.transpose(out=outT[:], in_=numS[:], identity=ident_t[:])
                    sS = fpool.tile([P, 1], F32)
                    nc.vector.tensor_scalar_max(out=sS[:], in0=scol[:], scalar1=1e-30)
                    rS = fpool.tile([P, 1], F32)
                    nc.vector.reciprocal(out=rS[:], in_=sS[:])
                    ob = fpool.tile([P, P], F32)
                    nc.vector.tensor_scalar(
                        out=ob[:],
                        in0=outT[:],
                        scalar1=rS[:],
                        scalar2=None,
                        op0=mybir.AluOpType.mult,
                    )
                    nc.sync.dma_start(out=out_ext[b * P : (b + 1) * P, :], in_=ob[:])

    nc.compile()
    return nc


def kernel(feat, beta, src, dst):
    feat = np.asarray(feat, dtype=np.float32)
    beta = np.asarray(beta, dtype=np.float32)
    src = np.asarray(src)
    dst = np.asarray(dst)
    in_maps, static = _prep(feat, beta, src, dst)
    nc = _build(static)
    res = run_bass_kernel_spmd(nc, in_maps, list(range(NCORES)))
    outs = [res.results[c]["out"][:NPC] for c in range(NCORES)]
    return np.concatenate(outs, axis=0).astype(np.float32)


if __name__ == "__main__":
    rng = np.random.default_rng(0)
    pass


# revision 30
# speedup vs baseline: 1.6524x; 1.6524x over previous
"""AGNNConv message-passing kernel for 8 TRN2 NeuronCores.

Sharding: edges sorted by destination; core c owns dst nodes
[12500c, 12500(c+1)).  Within a core, edges are grouped by
(dst-block of 128 nodes, src-chunk of 25088 nodes) and padded to a
multiple of 128; group sizes are uniform across cores (max over cores)
so all 8 cores run one SPMD graph.

Device graph per core:
  Phase A: L2-normalize the full node table to bf16 (4 src-chunk tables
           + own-dst-range table).
  Phase B: per piece, dma_gather src rows (S) and dst rows (Y);
           cos = reduce(S*Y); w = exp(beta*cos); masked one-hot matmuls
           accumulate num^T[d,j] and s[j] per dst block in PSUM;
           finalize out[j,:] = num[j,:] / s[j].
"""

import sys

sys.path.insert(0, "/opt/trn_rl_repo")

import ml_dtypes
import numpy as np

import concourse.bass as bass
import concourse.mybir as mybir
import concourse.tile as tile
from concourse import bacc
from concourse.bass_utils import run_bass_kernel_spmd
from concourse.library_config import mlp
from concourse.masks import make_identity

N, E, D = 100000, 3200000, 128
P = 128
NCORES = 8
NPC = 12500          # real dst nodes per core
NB = 98              # dst blocks per core (98*128 = 12544)
NPC_PAD = NB * P
NCHUNK = 4
CHUNK = 25088        # src chunk rows (196*128, < 2**15)
EPS = 1e-12
BPP = 3              # dst blocks per gather piece
UMAX = 16            # units (128 edges) per vector op group
F32 = mybir.dt.float32
BF16 = mybir.dt.bfloat16
I16 = mybir.dt.int16


def _pack_idx(arr_i16):
    """dma_gather index layout: idx i -> [i % 16, i // 16], replicated to
    128 partitions (8 copies of the 16-partition pattern)."""
    n = arr_i16.shape[0]
    l = arr_i16.reshape(n // 16, 16).T  # [16, n/16]
    return np.tile(l, (8, 1))  # [128, n/16]


def _prep(feat, beta, src, dst):
    """Host-side shard/index prep. Returns (in_maps, static) where static
    holds the compile-time structure shared by all cores."""
    nrm = np.sqrt((feat.astype(np.float32) ** 2).sum(1))  # [N]
    nrm = np.maximum(nrm, EPS)

    order = np.argsort(dst, kind="stable")
    ss = src[order].astype(np.int64)
    dd = dst[order].astype(np.int64)

    core = dd // NPC
    within = dd % NPC
    block = within // P
    jloc = (within % P).astype(np.float32)
    chunk = ss // CHUNK
    slocal = (ss % CHUNK).astype(np.int16)
    ylocal = within.astype(np.int16)

    comp = ((core * NB + block) * NCHUNK + chunk).astype(np.int64)
    counts = np.bincount(comp, minlength=NCORES * NB * NCHUNK).reshape(
        NCORES, NB, NCHUNK
    )
    Tbk = (np.ceil(counts.max(axis=0) / P) * P).astype(np.int64)  # [NB, NCHUNK]
    STk = Tbk.sum(axis=0)  # [NCHUNK] per-chunk stream length
    ST = int(STk.sum())
    NU = ST // P

    # edge order inside each (core, block, chunk) group
    order2 = np.argsort(comp, kind="stable")
    ss2, jl2, ch2, sl2, yl2 = (
        ss[order2],
        jloc[order2],
        chunk[order2],
        slocal[order2],
        ylocal[order2],
    )
    nrm_src = nrm[ss2].astype(np.float32)
    # start offset of each (c,b,k) group in the sorted edge array
    gstart = np.zeros(NCORES * NB * NCHUNK + 1, dtype=np.int64)
    np.cumsum(counts.reshape(-1), out=gstart[1:])

    # stream offsets: chunk-major streams, blocks in order inside each
    off_k = np.zeros((NCHUNK, NB + 1), dtype=np.int64)
    for k in range(NCHUNK):
        np.cumsum(Tbk[:, k], out=off_k[k, 1:])
    chunk_base = np.zeros(NCHUNK + 1, dtype=np.int64)
    np.cumsum(STk, out=chunk_base[1:])

    in_maps = []
    for c in range(NCORES):
        sidx = np.zeros(ST, dtype=np.int16)
        yidx = np.zeros(ST, dtype=np.int16)
        dloc = np.full(ST, 999.0, dtype=np.float32)
        wnrm = np.zeros(ST, dtype=np.float32)
        for k in range(NCHUNK):
            for b in range(NB):
                g = (c * NB + b) * NCHUNK + k
                cnt = gstart[g + 1] - gstart[g]
                if cnt == 0:
                    continue
                o = chunk_base[k] + off_k[k, b]
                sl_ = slice(gstart[g], gstart[g + 1])
                sidx[o : o + cnt] = sl2[sl_]
                yidx[o : o + cnt] = yl2[sl_]
                dloc[o : o + cnt] = jl2[sl_]
                wnrm[o : o + cnt] = 1.0 / nrm_src[sl_]

        lo = c * NPC
        fown = np.zeros((NPC_PAD, D), dtype=np.float32)
        fown[:NPC] = feat[lo : lo + NPC]

        in_maps.append(
            {
                "feat": np.ascontiguousarray(feat.astype(np.float32)),
                "fown": fown,
                "sidx": np.ascontiguousarray(_pack_idx(sidx)),
                "yidx": np.ascontiguousarray(_pack_idx(yidx)),
                "dloc": np.ascontiguousarray(dloc.reshape(NU, P).T),
                "wnrm": np.ascontiguousarray(wnrm.reshape(NU, P).T),
                "iota": np.ascontiguousarray(
                    np.tile(np.arange(P, dtype=np.float32), (P, 1))
                ).astype(ml_dtypes.bfloat16),
                "betar": np.full((P, 1), float(beta[0]), dtype=np.float32),
            }
        )

    static = dict(Tbk=Tbk, off_k=off_k, chunk_base=chunk_base, ST=ST, NU=NU)
    return in_maps, static


def _build(static):
    import os

    BISECT = os.environ.get("KBISECT", "")

    Tbk = static["Tbk"]
    off_k = static["off_k"]
    chunk_base = static["chunk_base"]
    ST = static["ST"]
    NU = static["NU"]

    nc = bacc.Bacc("TRN2")

    feat_ext = nc.declare_dram_parameter("feat", [N, D], F32, isOutput=False)
    fown_ext = nc.declare_dram_parameter("fown", [NPC_PAD, D], F32, isOutput=False)
    sidx_ext = nc.declare_dram_parameter("sidx", [P, ST // 16], I16, isOutput=False)
    yidx_ext = nc.declare_dram_parameter("yidx", [P, ST // 16], I16, isOutput=False)
    dloc_ext = nc.declare_dram_parameter("dloc", [P, NU], F32, isOutput=False)
    wnrm_ext = nc.declare_dram_parameter("wnrm", [P, NU], F32, isOutput=False)
    iota_ext = nc.declare_dram_parameter("iota", [P, P], BF16, isOutput=False)
    betar_ext = nc.declare_dram_parameter("betar", [P, 1], F32, isOutput=False)
    out_ext = nc.declare_dram_parameter("out", [NPC_PAD, D], F32, isOutput=True)

    htabs = [
        nc.dram_tensor(f"htab{k}", [CHUNK, D], BF16) for k in range(NCHUNK)
    ]
    hown = nc.dram_tensor("hown", [NPC_PAD, D], BF16)

    # piece structure over blocks
    pieces = [(b0, min(b0 + BPP, NB)) for b0 in range(0, NB, BPP)]
    LPK = np.zeros((len(pieces), NCHUNK), dtype=np.int64)
    for pi, (b0, b1) in enumerate(pieces):
        for k in range(NCHUNK):
            LPK[pi, k] = Tbk[b0:b1, k].sum()
    LMAX = int(LPK.max())

    with tile.TileContext(nc) as tc:
        with (
            tc.tile_pool(name="consts", bufs=1) as cpool,
            tc.tile_pool(name="pa", bufs=2) as papool,
            tc.tile_pool(name="pan", bufs=2) as pnpool,
            tc.tile_pool(name="gat", bufs=8) as gpool,
            tc.tile_pool(name="idx", bufs=8) as ipool,
            tc.tile_pool(name="vec", bufs=3) as vpool,
            tc.tile_pool(name="mw", bufs=4) as mpool,
            tc.tile_pool(name="fin", bufs=2) as fpool,
            tc.tile_pool(name="ps", bufs=2, space="PSUM") as pspool,
            tc.tile_pool(name="ps2", bufs=2, space="PSUM") as ps2pool,
        ):
            nc.gpsimd.load_library(mlp)

            iota_t = cpool.tile([P, P], BF16)
            nc.sync.dma_start(out=iota_t[:], in_=iota_ext[:])
            betar_t = cpool.tile([P, 1], F32)
            nc.sync.dma_start(out=betar_t[:], in_=betar_ext[:])
            ones_t = cpool.tile([P, 1], BF16)
            nc.vector.memset(ones_t[:], 1.0)
            ident_t = cpool.tile([P, P], F32)
            make_identity(nc, ident_t[:])

            # ---------------- Phase A: build bf16 tables ----------------
            # Rows are mapped partition-contiguously: partition p holds rows
            # [a + p*gn, a + (p+1)*gn) — 1 contiguous descriptor/partition.
            GA = 16  # row-tiles (x128 rows) per batched DMA

            def cast_range(src_ap, dst_ap, nrows):
                """plain f32 -> bf16 cast of nrows rows (raw src table)."""
                done = 0
                while done < nrows:
                    gn = min(GA, (nrows - done) // P)
                    if gn >= 1:
                        rows = gn * P
                        ft = papool.tile([P, GA, P], F32)
                        nc.sync.dma_start(
                            out=ft[:, :gn, :],
                            in_=src_ap[done : done + rows, :].rearrange(
                                "(p q) d -> p q d", p=P
                            ),
                        )
                        ht = papool.tile([P, GA, P], BF16)
                        nc.scalar.copy(out=ht[:, :gn, :], in_=ft[:, :gn, :])
                        nc.scalar.dma_start(
                            out=dst_ap[done : done + rows, :].rearrange(
                                "(p q) d -> p q d", p=P
                            ),
                            in_=ht[:, :gn, :],
                        )
                        done += rows
                    else:
                        rem = nrows - done
                        ft = papool.tile([P, P], F32)
                        nc.sync.dma_start(
                            out=ft[:rem, :], in_=src_ap[done:nrows, :]
                        )
                        ht = papool.tile([P, P], BF16)
                        nc.scalar.copy(out=ht[:rem, :], in_=ft[:rem, :])
                        nc.scalar.dma_start(
                            out=dst_ap[done:nrows, :], in_=ht[:rem, :]
                        )
                        done = nrows

            def conv_range(src_ap, dst_ap, nrows):
                """L2-normalize nrows rows (nrows % 128 == 0) to bf16."""
                done = 0
                while done < nrows:
                    gn = min(GA, (nrows - done) // P)
                    rows = gn * P
                    ft = papool.tile([P, GA, P], F32)
                    nc.sync.dma_start(
                        out=ft[:, :gn, :],
                        in_=src_ap[done : done + rows, :].rearrange(
                            "(p q) d -> p q d", p=P
                        ),
                    )
                    sqt = papool.tile([P, GA, P], F32)
                    nc.vector.tensor_tensor(
                        out=sqt[:, :gn, :],
                        in0=ft[:, :gn, :],
                        in1=ft[:, :gn, :],
                        op=mybir.AluOpType.mult,
                    )
                    sqcols = pnpool.tile([P, GA], F32)
                    nc.vector.tensor_reduce(
                        out=sqcols[:, :gn],
                        in_=sqt[:, :gn, :],
                        axis=mybir.AxisListType.X,
                        op=mybir.AluOpType.add,
                    )
                    nrmc = pnpool.tile([P, GA], F32)
                    nc.scalar.sqrt(out=nrmc[:, :gn], in_=sqcols[:, :gn])
                    invc = pnpool.tile([P, GA], F32)
                    nc.vector.tensor_scalar_max(
                        out=nrmc[:, :gn], in0=nrmc[:, :gn], scalar1=EPS
                    )
                    nc.vector.reciprocal(out=invc[:, :gn], in_=nrmc[:, :gn])
                    ht = papool.tile([P, GA, P], BF16)
                    for gi in range(gn):
                        nc.vector.tensor_scalar(
                            out=ht[:, gi, :],
                            in0=ft[:, gi, :],
                            scalar1=invc[:, gi : gi + 1],
                            scalar2=None,
                            op0=mybir.AluOpType.mult,
                        )
                    nc.scalar.dma_start(
                        out=dst_ap[done : done + rows, :].rearrange(
                            "(p q) d -> p q d", p=P
                        ),
                        in_=ht[:, :gn, :],
                    )
                    done += rows

            if BISECT != "z":
                for k in range(NCHUNK):
                    lo = k * CHUNK
                    nrows = min(CHUNK, N - lo)
                    cast_range(feat_ext[lo : lo + nrows, :], htabs[k][:], nrows)
                conv_range(fown_ext[:], hown[:], NPC_PAD)

            # ---------------- Phase B: edges ----------------
            # (Phase A -> B ordering is enforced by shadow-memory DRAM deps)
            if BISECT:
                zt = cpool.tile([P, P], F32)
                nc.vector.memset(zt[:], 0.0)
                for b in range(NB):
                    nc.sync.dma_start(
                        out=out_ext[b * P : (b + 1) * P, :], in_=zt[:]
                    )
            piece_list = [] if BISECT in ("a", "z") else pieces
            for pi, (b0, b1) in enumerate(piece_list):
                Sts, Yts, dlts, wnts = [], [], [], []
                ubase_piece = []
                for k in range(NCHUNK):
                    ln = int(LPK[pi, k])
                    if ln == 0:
                        Sts.append(None)
                        Yts.append(None)
                        dlts.append(None)
                        wnts.append(None)
                        ubase_piece.append(0)
                        continue
                    lu = ln // P
                    c0 = int((chunk_base[k] + off_k[k, b0]) // 16)
                    u0 = int((chunk_base[k] + off_k[k, b0]) // P)
                    ubase_piece.append(u0)
                    si = ipool.tile([P, LMAX // 16], I16)
                    nc.sync.dma_start(
                        out=si[:, : ln // 16], in_=sidx_ext[:, c0 : c0 + ln // 16]
                    )
                    yi = ipool.tile([P, LMAX // 16], I16)
                    nc.sync.dma_start(
                        out=yi[:, : ln // 16], in_=yidx_ext[:, c0 : c0 + ln // 16]
                    )
                    dl = ipool.tile([P, LMAX // P], F32)
                    nc.sync.dma_start(out=dl[:, :lu], in_=dloc_ext[:, u0 : u0 + lu])
                    wn = ipool.tile([P, LMAX // P], F32)
                    nc.sync.dma_start(out=wn[:, :lu], in_=wnrm_ext[:, u0 : u0 + lu])
                    St = gpool.tile([P, LMAX // P, P], BF16)
                    nc.gpsimd.dma_gather(
                        St[:, :lu, :],
                        htabs[k][:],
                        si[:, : ln // 16],
                        ln,
                        ln,
                        D,
                        single_packet=False,
                    )
                    Yt = gpool.tile([P, LMAX // P, P], BF16)
                    nc.gpsimd.dma_gather(
                        Yt[:, :lu, :],
                        hown[:],
                        yi[:, : ln // 16],
                        ln,
                        ln,
                        D,
                        single_packet=False,
                    )
                    Sts.append(St)
                    Yts.append(Yt)
                    dlts.append(dl)
                    wnts.append(wn)

                if BISECT == "g":
                    for k in range(NCHUNK):
                        if Sts[k] is not None:
                            nc.vector.tensor_copy(
                                out=Sts[k][:, 0, :], in_=Yts[k][:, 0, :]
                            )
                    continue
                for b in range(b0, b1):
                    numT = pspool.tile([P, P], F32)
                    scol = ps2pool.tile([P, 1], F32)
                    # count matmuls in this block for start/stop flags
                    nmm = sum(int(Tbk[b, k]) // P for k in range(NCHUNK))
                    mi = 0
                    for k in range(NCHUNK):
                        nub = int(Tbk[b, k]) // P
                        if nub == 0:
                            continue
                        St, Yt, dl, wn = Sts[k], Yts[k], dlts[k], wnts[k]
                        ub = int((chunk_base[k] + off_k[k, b]) // P) - ubase_piece[k]
                        for ug in range(0, nub, UMAX):
                            un = min(UMAX, nub - ug)
                            o = ub + ug
                            prod = vpool.tile([P, P], BF16)
                            dot = vpool.tile([P, UMAX], F32)
                            for u in range(un):
                                nc.vector.tensor_tensor_reduce(
                                    out=prod[:],
                                    in0=St[:, o + u, :],
                                    in1=Yt[:, o + u, :],
                                    scale=1.0,
                                    scalar=0.0,
                                    op0=mybir.AluOpType.mult,
                                    op1=mybir.AluOpType.add,
                                    accum_out=dot[:, u : u + 1],
                                )
                            t2 = vpool.tile([P, UMAX], F32)
                            nc.vector.tensor_tensor(
                                out=t2[:, :un],
                                in0=dot[:, :un],
                                in1=wn[:, o : o + un],
                                op=mybir.AluOpType.mult,
                            )
                            wexp = vpool.tile([P, UMAX], F32)
                            nc.scalar.activation(
                                out=wexp[:, :un],
                                in_=t2[:, :un],
                                func=mybir.ActivationFunctionType.Exp,
                                scale=betar_t[:, 0:1],
                            )
                            for u in range(un):
                                mw = mpool.tile([P, P], BF16)
                                nc.vector.tensor_scalar(
                                    out=mw[:],
                                    in0=iota_t[:],
                                    scalar1=dl[:, o + u : o + u + 1],
                                    scalar2=wexp[:, u : u + 1],
                                    op0=mybir.AluOpType.is_equal,
                                    op1=mybir.AluOpType.mult,
                                )
                                nc.tensor.matmul(
                                    out=numT[:],
                                    lhsT=St[:, o + u, :],
                                    rhs=mw[:],
                                    start=(mi == 0),
                                    stop=(mi == nmm - 1),
                                    skip_group_check=True,
                                )
                                nc.tensor.matmul(
                                    out=scol[:],
                                    lhsT=mw[:],
                                    rhs=ones_t[:],
                                    start=(mi == 0),
                                    stop=(mi == nmm - 1),
                                    skip_group_check=True,
                                )
                                mi += 1
                    # finalize block b
                    numS = fpool.tile([P, P], F32)
                    nc.scalar.copy(out=numS[:], in_=numT[:])
                    outT = pspool.tile([P, P], F32)
                    nc.tensor.transpose(out=outT[:], in_=numS[:], identity=ident_t[:])
                    sS = fpool.tile([P, 1], F32)
                    nc.vector.tensor_scalar_max(out=sS[:], in0=scol[:], scalar1=1e-30)
                    rS = fpool.tile([P, 1], F32)
                    nc.vector.reciprocal(out=rS[:], in_=sS[:])
                    ob = fpool.tile([P, P], F32)
                    nc.vector.tensor_scalar(
                        out=ob[:],
                        in0=outT[:],
                        scalar1=rS[:],
                        scalar2=None,
                        op0=mybir.AluOpType.mult,
                    )
                    nc.sync.dma_start(out=out_ext[b * P : (b + 1) * P, :], in_=ob[:])

    nc.compile()
    return nc


def kernel(feat, beta, src, dst):
    feat = np.asarray(feat, dtype=np.float32)
    beta = np.asarray(beta, dtype=np.float32)
    src = np.asarray(src)
    dst = np.asarray(dst)
    in_maps, static = _prep(feat, beta, src, dst)
    nc = _build(static)
    res = run_bass_kernel_spmd(nc, in_maps, list(range(NCORES)))
    outs = [res.results[c]["out"][:NPC] for c in range(NCORES)]
    return np.concatenate(outs, axis=0).astype(np.float32)


if __name__ == "__main__":
    rng = np.random.default_rng(0)
    pass
